# revision 58
# baseline (speedup 1.0000x reference)
"""Multi-head attention (B=2, N=2048, C=1024, H=16) on 8 trn2 NeuronCores.

Tensor-parallel over heads: core c computes heads {2c, 2c+1} for both batch
elements and emits a partial output y_c = attn_out_c @ W_out[local rows]
(bf16 partials); the host sums the 8 partials and adds b_out.

v2 schedule notes (measured ~218us vs 222us baseline; both engines near
saturation: PE busy ~185us of matmul streams+drains, ScalarE ~153us):

  - The exp stream (128 ACTIVATEs x ~1.11us back-to-back) runs only on
    ScalarE (1 elem/cycle/lane @1.2GHz, +352cyc ramp/instr). The PE is
    the actual critical engine: S 27us + PV 55us + QKV 41us + V-trans
    9us + proj 14us + exposed drains/stalls.
  - There is NO elasticity between PE and ScalarE beyond the 2-buffer
    ps_st pool (S(kc+2) WARs exp(kc)): any slot whose PE work exceeds
    the 1.11us exp cadence leaks ACT idle that is never recovered, so
    QKV work is spread as ~0.45us 2-matmul quarter-units on a static
    (q, kc) due-slot schedule, sized to per-slot slack. q0 of batch 0
    is inherently over-packed (hard just-in-time deadlines: K_w must
    close before slot 4w-2 EMITS the S(4w) lookahead -- a consumer
    emitted before its producer binds stale SBUF, silently).
  - ACT table preload via a dummy exp at t~0; PE warmup (28 identity
    matmuls, ~3.6us continuous -- shorter bursts never trip the HAM
    window) un-throttles the 1.2GHz cold clock before the first chains.
  - Head DMAs: x is packed partition-major in DRAM so each 512-token
    window is ONE contiguous 1MB DMA; DMA durations in traces are only
    descriptor issue -- transfers land ~8-10us later when prefetch
    competes, so the K/Q-critical set is issued first on all three DMA
    queues (sync/scalar/gpsimd) with prefetch strictly behind.
  - Output projections are deferred into a FIFO and drained by a credit
    pacer (cap 2/slot; never while a misc accumulator chain is open --
    the 2-buffer misc pool would deadlock the in-order PE queue; never
    at kc 15; 1/slot forced in the final q-chunk -- more jams the DVE
    and backs the misc pool into the PE).
  - Cross-boundary: S(next,0) is emitted BEFORE the last PV pair (it
    only WARs exp(q,14)) so exp(next,0) queues with no bubble.
  - Tail: final-boundary PSUM evictions on ScalarE (free after the last
    exp), keep-warm identity matmuls through the final norm, final
    y stores split across two DMA queues.

Per-core pipeline details kept from v1: S^T = K @ Q^T as a row-tiled
pair (head h in rows h*64..) streaming concurrently; P^T = exp(S^T/32)
on ScalarE straight from PSUM ([128, 1024] ops); PV via ones-augmented V
(65th stationary column accumulates softmax denominators); V transposed
on the PE; normalization via DVE reciprocal + GpSimd partition_broadcast;
projection all-bf16. PSUM: st 2x2 banks + pv 2 + misc 2 = 8 (full).
Never emit a consumer before its producer: per-engine emission order IS
program order.

absmax error ~5.2e-3 of the output scale vs the fp32 reference (bf16
operands; fp8 P/V was simulated and REJECTED: P e4m3 alone gives 2.7e-2,
over the 2e-2 gate; e3m4 gets no DoubleRow speedup so fp8 PV is dead).
"""
import os
import sys

sys.path.insert(0, "/opt/trn_rl_repo")

import ml_dtypes
import numpy as np

import concourse.bacc as bacc
import concourse.mybir as mybir
import concourse.tile as tile
from concourse import bass_utils
from concourse.masks import make_identity

F32 = mybir.dt.float32
BF16 = mybir.dt.bfloat16
NPBF16 = ml_dtypes.bfloat16

EMB = 1024
HEADS = 16
B = 2
SEQ = 2048
D = 64
NCORES = 8
HPC = HEADS // NCORES          # heads per core = 2
LD = HPC * D                   # local head dim = 128
TSEQ = B * SEQ                 # 4096
CC = EMB // 128                # contraction chunks = 8
SCALE = float(EMB) ** -0.5     # 1/32

QCH = 512                      # q chunk (free dim of S^T matmuls)
NQ = SEQ // QCH                # 4 q-chunks per batch
NK = SEQ // 128                # 16 k-chunks per batch
NW = TSEQ // 512               # 8 x windows (4 per batch)

# approximate PE costs (us) for the credit pacer
C_SPAIR = 0.22                 # S^T pair (row-tiled, concurrent)
C_PVPAIR = 0.44                # PV pair (2 serial N=512 matmuls)
C_EXP = 1.00                   # one [128,1024] ACTIVATE (queued cadence)
C_QKVP = 0.88                  # one qkv half-chain (4 N=512 matmuls)
C_PT = 1.15                    # V transpose unit (4 PE transposes)
C_PROJ = 0.22                  # one projection matmul


def _build():
    nc = bacc.Bacc("TRN2", target_bir_lowering=False, debug=False,
                   num_devices=NCORES)

    # x packed partition-major: xP[p, w, kc, c] = x-emb (kc*128+p) of
    # token (w*512+c). Each window is one fully-contiguous 1MB DMA.
    xP = nc.dram_tensor("xP", [128, NW, CC, 512], BF16,
                        kind="ExternalInput")
    # tensor-major: [k | q | v], each [128, CC*LD]
    wqkv = nc.dram_tensor("wqkv", [128, 3 * CC * LD], BF16,
                          kind="ExternalInput")
    bqkv = nc.dram_tensor("bqkv", [LD, 3], F32, kind="ExternalInput")
    wout = nc.dram_tensor("wout", [LD, EMB], BF16, kind="ExternalInput")
    y = nc.dram_tensor("y", [TSEQ // 128, 128, EMB], BF16,
                       kind="ExternalOutput")
    # tiny scratch output used only as a DMA ordering gate (its write
    # depends on QT window 0, so prefetch queued behind it cannot steal
    # HBM bandwidth from the head's critical transfers)
    scratch = nc.dram_tensor("scratch", [1, 8], BF16,
                             kind="ExternalOutput")

    xP_c = xP.ap()
    wqkv_c = wqkv.ap()

    with tile.TileContext(nc) as tc:
        with (
            tc.tile_pool(name="persist", bufs=1) as persist,
            tc.tile_pool(name="vt", bufs=2) as vtp,
            tc.tile_pool(name="psb", bufs=8) as psb,
            tc.tile_pool(name="norm", bufs=3) as normp,
            tc.tile_pool(name="yout", bufs=10) as youtp,
            tc.tile_pool(name="ps_st", bufs=2, space="PSUM") as ps_st,
            tc.tile_pool(name="ps_pv", bufs=1, space="PSUM") as ps_pv,
            tc.tile_pool(name="ps_misc", bufs=2, space="PSUM") as ps_misc,
        ):
            # -------- head: ACT table preload + DMAs spread over queues
            dummy = persist.tile([1, 8], F32, tag="dummy")
            dummy_o = persist.tile([1, 8], F32, tag="dummy_o")
            nc.vector.memset(dummy[:], 0.0)
            nc.scalar.activation(dummy_o[:], dummy[:],
                                 mybir.ActivationFunctionType.Exp,
                                 scale=SCALE)

            bqkv_sb = persist.tile([LD, 3], F32, tag="bqkv")
            nc.scalar.dma_start(bqkv_sb[:], bqkv.ap())

            wall = persist.tile([128, 3 * CC * LD], BF16, tag="wall")
            WT = CC * LD  # 1024 cols per tensor
            # HBM bandwidth is the head's critical resource and DMA
            # transfers are served roughly in issue order per queue:
            # everything the K/Q chains need goes FIRST on each queue,
            # prefetch strictly behind.
            nc.sync.dma_start(wall[:, 0:WT], wqkv_c[:, 0:WT])
            nc.scalar.dma_start(wall[:, WT:2 * WT], wqkv_c[:, WT:2 * WT])

            # per-window x tiles [128, CC, 512]; xfull[kc, w] slices one
            # contraction chunk.
            xw = [persist.tile([128, CC, 512], BF16, tag=f"xw{w}",
                               name=f"xw{w}") for w in range(NW)]
            xfull = {(kc, w): xw[w][:, kc]
                     for w in range(NW) for kc in range(CC)}

            # identity first: DVE memset + gpsimd affine_select BEFORE any
            # gpsimd DMA, so warmup matmuls can start at ~7.5us.
            ident = persist.tile([128, 128], BF16, tag="ident")
            nc.vector.memset(ident[:], 0.0)
            nc.gpsimd.affine_select(
                out=ident[:], in_=ident[:],
                compare_op=mybir.AluOpType.not_equal,
                fill=1.0, base=0, pattern=[[-1, 128]],
                channel_multiplier=1)

            # window 0 split across sync (kc 0-3, behind wk) and gpsimd
            # (kc 4-7, behind the affine_select)
            nc.sync.dma_start(xw[0][:, 0:4], xP_c[:, 0, 0:4])
            nc.gpsimd.dma_start(xw[0][:, 4:8], xP_c[:, 0, 4:8])

            # -------- PE warmup (HAM un-throttle): un-chained N=128
            # matmuls (each misc tile takes 4 writes to disjoint
            # quarters, so subtile deps leave them back-to-back) while
            # the head DMAs land; HAM's SHORT window then fires and the
            # real chains run at 2.4GHz.
            # ~28 matmuls x ~0.13us = ~3.6us busy, enough to cover the
            # HAM SHORT window (a shorter burst never un-throttles).
            for i in range(7):
                wm = ps_misc.tile([128, 512], F32, tag="misc")
                for j in range(4):
                    nc.tensor.matmul(wm[:, j * 128:(j + 1) * 128],
                                     ident[:], ident[:],
                                     start=True, stop=True)

            # prefetch, behind the critical head set: V weights + wout
            # (needed at ~16us), then batch-0's x windows as ONE
            # contiguous 1MB DMA each. ScalarE's queue stays clear for
            # the exp stream.
            nc.gpsimd.dma_start(wall[:, 2 * WT:3 * WT],
                                wqkv_c[:, 2 * WT:3 * WT])
            wout_sb = persist.tile([LD, EMB], BF16, tag="wout")
            nc.gpsimd.dma_start(wout_sb[:], wout.ap())
            for w in (1, 2, 3):
                nc.sync.dma_start(xw[w][:], xP_c[:, w])

            w_sb = {}
            for i, nm in enumerate(("k", "q", "v")):
                for kc in range(CC):
                    w_sb[nm, kc] = wall[:, i * WT + kc * LD:
                                        i * WT + (kc + 1) * LD]
            bias_sb = {nm: bqkv_sb[:, i:i + 1]
                       for i, nm in enumerate(("q", "k", "v"))}

            # persistent activations (per batch)
            QT = [persist.tile([LD, SEQ], BF16, tag=f"QT{b}",
                               name=f"QT{b}") for b in range(B)]
            KT = [persist.tile([LD, SEQ], BF16, tag=f"KT{b}", name=f"KT{b}")
                  for b in range(B)]
            outT = [persist.tile([LD, SEQ], BF16, tag=f"outT{b}",
                                 name=f"outT{b}") for b in range(B)]
            # vaug[b,kc][:, h, 0:64] = V^T chunk for head h; [:, h, 64] = 1
            # (ones-memsets are emitted later, after the critical K/Q
            # bias-adds, to keep the DVE queue clear in the head)
            vaug = {}
            for b in range(B):
                for kc in range(NK):
                    vaug[b, kc] = persist.tile([128, 2, 66], BF16,
                                               tag=f"vaug{b}_{kc}",
                                               name=f"vaug{b}_{kc}")

            # ---------------- building blocks --------------------------
            def qkv_parts(b, sc, nm):
                """Unit list [(fn, pe_cost, kind)] for one (batch,
                window, tensor) projection, split into ~0.45us 2-matmul
                quarters so per-slot PE load stays near-constant (big
                per-slot variance oscillates the shallow exp pipeline).
                kind: 'open' holds the misc accumulator, 'mid' continues
                it, 'close' finishes it; 'pt' must precede this slot's
                PV read of vaug."""
                s0 = sc * 512
                w = b * 4 + sc
                cell = {}

                def quarter(j):
                    def f():
                        if j == 0:
                            cell["ps"] = ps_misc.tile([128, 512], F32,
                                                      tag="misc",
                                                      name="qkv_ps")
                        ps = cell["ps"]
                        for kc in range(2 * j, 2 * j + 2):
                            nc.tensor.matmul(
                                ps[:], w_sb[nm, kc], xfull[kc, w][:],
                                start=(kc == 0), stop=(kc == CC - 1))
                        if j < 3:
                            return
                        if nm == "q":
                            nc.vector.tensor_scalar_add(
                                QT[b][:, s0:s0 + 512], ps[:],
                                bias_sb["q"])
                        elif nm == "k":
                            nc.vector.tensor_scalar_add(
                                KT[b][:, s0:s0 + 512], ps[:],
                                bias_sb["k"])
                        else:
                            vt = vtp.tile([128, 512], BF16, tag="vt")
                            nc.vector.tensor_scalar_add(vt[:], ps[:],
                                                        bias_sb["v"])
                            cell["vt"] = vt
                    return f

                def pt_half(j0):
                    def f():
                        vt = cell["vt"]
                        if j0 == 0:
                            cell["pst4"] = ps_misc.tile(
                                [128, 4, 2, D], BF16, tag="misc",
                                name="pst4")
                        pst4 = cell["pst4"]
                        for j in (j0, j0 + 1):
                            nc.tensor.transpose(
                                pst4[:, j], vt[:, j * 128:(j + 1) * 128],
                                ident[:])
                        for j in (j0, j0 + 1):
                            nc.vector.tensor_copy(
                                vaug[b, sc * 4 + j][:, :, 0:D],
                                pst4[:, j])
                    return f

                units = [(quarter(0), C_QKVP / 2, "open"),
                         (quarter(1), C_QKVP / 2, "mid"),
                         (quarter(2), C_QKVP / 2, "mid"),
                         (quarter(3), C_QKVP / 2, "close")]
                if nm == "v":
                    units += [(pt_half(0), C_PT / 2, "pt_open"),
                              (pt_half(2), C_PT / 2, "pt_close")]
                return units

            pre_pts = {}

            def st_exp(b, q, kc):
                """S^T pair + exp for (batch, q-chunk, k-chunk)."""
                q0 = q * QCH
                st = ps_st.tile([128, 2 * QCH], F32, tag="st")
                k0 = kc * 128
                for h in range(HPC):
                    nc.tensor.matmul(
                        st[:, h * QCH:(h + 1) * QCH],
                        KT[b][h * D:(h + 1) * D, k0:k0 + 128],
                        QT[b][h * D:(h + 1) * D, q0:q0 + QCH],
                        start=True, stop=True)
                pt = psb.tile([128, 2 * QCH], BF16, tag="pt")
                nc.scalar.activation(pt[:], st[:],
                                     mybir.ActivationFunctionType.Exp,
                                     scale=SCALE)
                return pt

            pending = []

            def proj_unit(b, sc, n, eng=None, evict_eng=None,
                          split=False, psrc=None):
                rt = b * (SEQ // 128) + sc
                pool, ptag = psrc or (ps_misc, "misc")
                ps = pool.tile([128, 512], F32, tag=ptag, name="proj_ps")
                nc.tensor.matmul(
                    ps[:], outT[b][:, sc * 128:(sc + 1) * 128],
                    wout_sb[:, n * 512:(n + 1) * 512],
                    start=True, stop=True)
                yt = youtp.tile([128, 512], BF16, tag="yt")
                if evict_eng is nc.scalar:
                    nc.scalar.copy(yt[:], ps[:])
                else:
                    nc.vector.tensor_copy(yt[:], ps[:])
                if eng is None:
                    eng = nc.gpsimd if (sc + n) % 2 else nc.sync
                if split:
                    # halve the store so the final drain parallelizes
                    # across two queues
                    oth = nc.sync if eng is nc.gpsimd else nc.gpsimd
                    eng.dma_start(
                        y.ap()[rt, :, n * 512:n * 512 + 256],
                        yt[:, 0:256])
                    oth.dma_start(
                        y.ap()[rt, :, n * 512 + 256:(n + 1) * 512],
                        yt[:, 256:512])
                else:
                    eng.dma_start(
                        y.ap()[rt, :, n * 512:(n + 1) * 512], yt[:])

            # ---------------- attention phase ---------------------------
            def phase(b, due, final=False):
                """Attention for batch b. `due` maps (q, kc) -> unit list
                force-injected at that slot. Deferred projections drain
                via a credit pacer, never while a misc chain is open."""
                credit = 0.0
                guard = [0]

                def run_unit(u):
                    fn, cost, kind = u
                    if kind in ("open", "pt_open"):
                        guard[0] += 1
                    elif kind in ("close", "pt_close"):
                        guard[0] -= 1
                    fn()
                    return cost

                for q in range(NQ):
                    q0 = q * QCH
                    if q + 1 < NQ:
                        nxt = (b, q + 1)
                    elif b + 1 < B:
                        nxt = (b + 1, 0)
                    else:
                        nxt = None
                    pvs = [ps_pv.tile([D + 1, QCH], F32, tag=f"pv{h}",
                                      name=f"pv{h}") for h in range(HPC)]
                    for kc in range(NK):
                        pt = pre_pts.pop((b, q, kc), None)
                        if pt is None:
                            pt = st_exp(b, q, kc)
                        credit += C_EXP - C_PVPAIR
                        # forced due units (QKV chains / V transposes):
                        # 'pre' units run BEFORE this slot's S-pair
                        # emission (producers for the lookahead's KT/QT
                        # reads and this slot's PV read of vaug MUST be
                        # emitted first -- later-emitted producers bind
                        # consumers to stale SBUF), 'post' units go
                        # after the PV pair where their ready matmuls
                        # absorb the next stall.
                        slot = due.pop((q, kc), None) or {}
                        for u in slot.get("pre", ()):
                            credit -= run_unit(u)
                        dues = slot.get("post", ())
                        # two-deep S^T/exp lookahead
                        for ahead in (1, 2):
                            nkc = kc + ahead
                            if nkc < NK and (b, q, nkc) not in pre_pts:
                                pre_pts[b, q, nkc] = st_exp(b, q, nkc)
                                credit -= C_SPAIR
                        # cross-boundary: S(next,0) BEFORE the last PV
                        # pair -- it only WARs exp(q,14), so it streams
                        # during exp(q,15) and exp(next,0) queues with
                        # zero bubble.
                        if kc == NK - 1 and nxt is not None:
                            pre_pts[nxt[0], nxt[1], 0] = st_exp(
                                nxt[0], nxt[1], 0)
                            credit -= C_SPAIR
                        for h in range(HPC):
                            nc.tensor.matmul(
                                pvs[h][:],
                                vaug[b, kc][:, h, 0:D + 1],
                                pt[:, h * QCH:(h + 1) * QCH],
                                start=(kc == 0), stop=(kc == NK - 1))
                        for u in dues:
                            credit -= run_unit(u)
                        # paced projection drain. At kc 0/1 the PE is
                        # guaranteed to stall on the exp queue crossing
                        # the q-boundary, and in the final q-chunk the
                        # backlog must empty before the flush -- drain
                        # there regardless of credit (1/slot: more jams
                        # DVE and backs the misc pool into the PE).
                        force = final and q == NQ - 1
                        cap = 1 if (final and q == NQ - 1) else 2
                        if kc < 15 and not guard[0]:
                            npop = 0
                            while (pending and npop < cap
                                   and (credit >= C_PROJ or force)):
                                pending.pop(0)()
                                credit -= C_PROJ
                                npop += 1
                    # second cross-boundary pair (WARs exp(q,15))
                    if nxt is not None:
                        pre_pts[nxt[0], nxt[1], 1] = st_exp(
                            nxt[0], nxt[1], 1)
                        credit -= C_SPAIR
                    # normalize straight from PSUM: reciprocal reads the
                    # denominator row and the multiply reads the value
                    # rows in place (no eviction copies -- halves the
                    # per-boundary DVE chain; the pv banks are held
                    # until the muls, same release point as before).
                    last = final and q == NQ - 1
                    rcss, pes = [], []
                    for h in range(HPC):
                        ss = normp.tile([1, QCH], F32, tag="ss",
                                        name=f"ss{h}")
                        nc.vector.tensor_copy(ss[:], pvs[h][D:D + 1, :])
                        rcs = normp.tile([1, QCH], F32, tag="rcs",
                                         name=f"rcs{h}")
                        nc.vector.reciprocal_approx_fast(rcs[:], ss[:])
                        rcss.append(rcs)
                        pe = normp.tile([D, QCH], BF16, tag="pe",
                                        name=f"pe{h}")
                        if last:
                            # ScalarE is free after the final exp: take
                            # the big evictions off the serial DVE chain
                            nc.scalar.copy(pe[:], pvs[h][0:D, :])
                        else:
                            nc.vector.tensor_copy(pe[:], pvs[h][0:D, :])
                        pes.append(pe)
                    if last:
                        # keep the PE's HAM clock warm through the final
                        # norm so the closing projections run at 2.4GHz
                        for i in range(12):
                            wst = ps_st.tile([128, 2 * QCH], F32,
                                             tag="st", name="warm_st")
                            nc.tensor.matmul(wst[:, 0:128], ident[:],
                                             ident[:], start=True,
                                             stop=True)
                    # drain stale projections while DVE normalizes
                    # (their outT inputs are from older q-chunks: no PE
                    # wait; the PE is stalling here anyway, so ignore
                    # credit)
                    for _ in range(2):
                        if pending and not guard[0]:
                            pending.pop(0)()
                            credit -= C_PROJ
                    for h in range(HPC):
                        rb = normp.tile([D, QCH], F32, tag="rb")
                        nc.gpsimd.partition_broadcast(rb[:], rcss[h][:])
                        nc.vector.tensor_mul(
                            outT[b][h * D:(h + 1) * D, q0:q0 + QCH],
                            pes[h][:], rb[:])
                    pending.extend(
                        (lambda b=b, sc=sc, n=n, **kw:
                         proj_unit(b, sc, n, **kw))
                        for sc in range(4 * q, 4 * q + 4)
                        for n in range(EMB // 512))

            # ---------------- emission program --------------------------
            # b0 win0: K then Q chains, then seed S(0,0)/S(0,1) so exp
            # starts ASAP; V win0 is forced at the first loop slot.
            for u in qkv_parts(0, 0, "k"):
                u[0]()
            for u in qkv_parts(0, 0, "q"):
                u[0]()
            # batch-1's x windows prefetch only after QT window 0 exists
            # (the gate DMA reads QT, so these 4MB of transfers cannot
            # compete with the head's critical DMAs)
            nc.gpsimd.dma_start(scratch.ap(), QT[0][0:1, 0:8])
            for w in (4, 5, 6, 7):
                nc.gpsimd.dma_start(xw[w][:], xP_c[:, w])
            pre_pts[0, 0, 0] = st_exp(0, 0, 0)
            pre_pts[0, 0, 1] = st_exp(0, 0, 1)
            # vaug ones columns (DVE queue is clear of critical adds now)
            for b in range(B):
                for kc in range(NK):
                    nc.vector.memset(vaug[b, kc][:, :, 64:65], 1.0)

            def put(due, q, kc, units, pre=False):
                slot = due.setdefault((q, kc), {"pre": [], "post": []})
                slot["pre" if pre else "post"].extend(units)

            # batch-0 loop schedule at quarter-unit granularity (~0.45us
            # each) so per-slot PE load stays near-constant. Deadlines:
            # K_w must CLOSE before slot (4w-2)'s lookahead EMITS
            # S(0,0,4w) (post of slot 4w-3 at the latest); V_w's
            # transposes go 'pre' at the PV slots that read them.
            due0 = {}
            V0 = qkv_parts(0, 0, "v")
            K1 = qkv_parts(0, 1, "k")
            V1 = qkv_parts(0, 1, "v")
            K2 = qkv_parts(0, 2, "k")
            V2 = qkv_parts(0, 2, "v")
            K3 = qkv_parts(0, 3, "k")
            V3 = qkv_parts(0, 3, "v")
            Q1 = qkv_parts(0, 1, "q")
            put(due0, 0, 0, V0[0:5], pre=True)
            put(due0, 0, 0, K1[0:2])
            put(due0, 0, 1, K1[2:4])
            put(due0, 0, 2, [V0[5]], pre=True)
            put(due0, 0, 3, V1[0:3])
            put(due0, 0, 4, V1[3:5], pre=True)
            put(due0, 0, 4, K2[0:2])
            put(due0, 0, 5, K2[2:4])
            put(due0, 0, 6, [V1[5]], pre=True)
            put(due0, 0, 7, V2[0:3])
            put(due0, 0, 8, V2[3:5], pre=True)
            put(due0, 0, 8, K3[0:2])
            put(due0, 0, 9, K3[2:4])
            put(due0, 0, 10, [V2[5]], pre=True)
            put(due0, 0, 11, V3[0:3])
            put(due0, 0, 12, V3[3:5], pre=True)
            put(due0, 0, 13, Q1[0:2])
            put(due0, 0, 14, [V3[5]], pre=True)
            put(due0, 0, 14, Q1[2:4])

            def seq(due, q, kc, units):
                for u in units:
                    put(due, q, kc, [u])
                    kc += 1
                    if kc == 15:
                        q, kc = q + 1, 0
                return q, kc

            # b0's Q2/Q3 close well before their boundary pre-issues;
            # then batch-1's early windows, one unit per slot (their
            # consumers are in batch-1's own loop, far away).
            pos = seq(due0, 1, 1, qkv_parts(0, 2, "q"))
            pos = seq(due0, *pos, qkv_parts(0, 3, "q"))
            for nm, w in [("k", 0), ("q", 0), ("v", 0), ("k", 1),
                          ("v", 1), ("k", 2), ("v", 2)]:
                pos = seq(due0, *pos, qkv_parts(1, w, nm))

            # batch-1 late windows stream inside batch-1's own loop.
            # K3 closes at slot 3, before slot 10's lookahead emits
            # S(1,0,12); V3's transposes land before PV(12)/PV(14).
            due1 = {}
            pos = seq(due1, 0, 0, qkv_parts(1, 3, "k"))
            pos = seq(due1, *pos, qkv_parts(1, 3, "v"))
            pos = seq(due1, *pos, qkv_parts(1, 1, "q"))
            pos = seq(due1, 1, 0, qkv_parts(1, 2, "q"))
            pos = seq(due1, 2, 0, qkv_parts(1, 3, "q"))

            phase(0, due0)
            phase(1, due1, final=True)

            # flush remaining projections (b1 q3's 8 units); ScalarE is
            # free after the last exp, so it takes half the PSUM
            # evictions (halving the serial DVE chain) and a DMA share.
            # The st and pv PSUM pools are idle at the flush: rotating
            # the projection accumulators across all three pools stops
            # the 2-buffer misc pool's eviction WAR from pacing the
            # final matmuls.
            engs = [nc.sync, nc.gpsimd, nc.scalar]
            pools = [(ps_misc, "misc"), (ps_st, "st"), (ps_pv, "pv0"),
                     (ps_misc, "misc"), (ps_st, "st"), (ps_pv, "pv1")]
            for j, p in enumerate(pending):
                p(eng=engs[j % 3],
                  evict_eng=nc.scalar if j % 2 else nc.vector,
                  split=True, psrc=pools[j % 6])
            del pending[:]

    nc.compile()
    return nc


_NC = None


def _get_nc():
    global _NC
    if _NC is None:
        _NC = _build()
    return _NC


def kernel(x, W_qkv, b_qkv, W_out, b_out):
    x = np.asarray(x, dtype=np.float32)
    W_qkv = np.asarray(W_qkv, dtype=np.float32)
    b_qkv = np.asarray(b_qkv, dtype=np.float32)
    W_out = np.asarray(W_out, dtype=np.float32)
    b_out = np.asarray(b_out, dtype=np.float32)

    nc = _get_nc()

    xT = x.reshape(TSEQ, EMB).T.astype(NPBF16).reshape(CC, 128, NW, 512)
    xPh = np.ascontiguousarray(xT.transpose(1, 2, 0, 3))  # [128,NW,CC,512]
    Wr = W_qkv.reshape(EMB, 3, HEADS, D)
    br = b_qkv.reshape(3, HEADS, D)

    in_maps = []
    for c in range(NCORES):
        h0, h1 = HPC * c, HPC * (c + 1)
        # weights tensor-major [k | q | v], each [128, CC*LD] chunk-major
        wt = np.stack(
            [Wr[:, i, h0:h1].reshape(CC, 128, LD) for i in (1, 0, 2)],
            axis=0)                       # [3(kqv), CC, 128, LD]
        wt = wt.transpose(2, 0, 1, 3).reshape(128, 3 * CC * LD)
        in_maps.append({
            "xP": xPh,
            "wqkv": np.ascontiguousarray(wt).astype(NPBF16),
            # bias columns stay (q, k, v) as in v1
            "bqkv": np.ascontiguousarray(
                np.stack([br[i, h0:h1].reshape(LD) for i in range(3)],
                         axis=1)),
            "wout": W_out[LD * c:LD * (c + 1)].astype(NPBF16),
        })

    res = bass_utils.run_bass_kernel_spmd(
        nc, in_maps, core_ids=list(range(NCORES)), trace=False)

    acc = np.zeros((TSEQ // 128, 128, EMB), dtype=np.float64)
    for c in range(NCORES):
        acc += res.results[c]["y"].astype(np.float64)
    out = (acc.reshape(TSEQ, EMB) + b_out).astype(np.float32)
    return out.reshape(B, SEQ, EMB)
